# revision 7
# baseline (speedup 1.0000x reference)
"""Trainium2 Bass kernel for nn_Contracter (e3nn tensor product + message passing).

  reference:  x2_scatter = segment_sum(x2, idxs, N); x2g = x2_scatter[idxs]
              out[e,u,k] = sum_ij x1[e,u,i] * x2g[e,u,j] * ww3j[u,i,j,k]
              ww3j[u,i,j,k] = sum_p weights[u,p] * w3j[p,i,j,k]

  Strategy: sort edges by node; shard nodes (and their edges) across the 8
  cores, so each core's segment-sum is fully local (no collectives).  On
  each core:
    sweep 1:  per 128-node block, accumulate table[n,(u,j)] with one-hot
              matmuls over that block's (padded) edge chunks.
    mid:      PE-transpose table -> tableT; build per-node contraction
              table  Ctable[n,(u,k,i)] = sum_j table[n,(u,j)]*ww3j[u,i,j,k]
              with block-diagonal weight matmuls (tiny: nodes only).
    sweep 2:  per edge chunk: gather CG = onehotT @ Ctable (PE), multiply
              by x1 broadcast over k (DVE, bf16), reduce over i, DMA out.
  All matmuls run in fp32r (~13-bit mantissa); products in bf16.
"""
import sys
sys.path.insert(0, "/opt/trn_rl_repo")
import numpy as np
import concourse.bass as bass
import concourse.bacc as bacc
import concourse.mybir as mybir
import concourse.tile as tile
from concourse import bass_utils
from concourse.masks import make_identity

P = 128
E = 100_000
N = 10_000
NCORES = 8
MUL, BD = 32, 9
DIM = MUL * BD            # 288
CDIM = MUL * BD * BD      # 2592
f32 = mybir.dt.float32
f32r = mybir.dt.float32r
bf16 = mybir.dt.bfloat16

# u-groups for block-diagonal weight matmuls (rows (u,j) <= 128)
UGROUPS = [(0, 11), (11, 22), (22, 32)]

_CACHE = {}


# ----------------------------------------------------------------- host prep
def _plan(idxs):
    order = np.argsort(idxs, kind="stable")
    deg = np.bincount(idxs, minlength=N)
    cum = np.concatenate([[0], np.cumsum(deg)])
    n_bounds = [0]
    for c in range(1, NCORES):
        n_bounds.append(int(np.searchsorted(cum, c * len(idxs) / NCORES)))
    n_bounds.append(N)
    cores = [dict(n_lo=n_bounds[c], n_hi=n_bounds[c + 1]) for c in range(NCORES)]
    NB = int(np.ceil(max(cr["n_hi"] - cr["n_lo"] for cr in cores) / P))
    CPB = np.zeros(NB, dtype=int)
    for cr in cores:
        n_lo, n_hi = cr["n_lo"], cr["n_hi"]
        for b in range(NB):
            blo, bhi = n_lo + b * P, min(n_lo + (b + 1) * P, n_hi)
            cnt = int(cum[bhi] - cum[blo]) if blo < n_hi else 0
            CPB[b] = max(CPB[b], (cnt + P - 1) // P)
    CPB = np.maximum(CPB, 1)
    return dict(order=order, cum=cum, cores=cores, NB=NB, CPB=CPB,
                E_pad=int(P * CPB.sum()))


def _core_arrays(plan, idxs, x1, x2):
    NB, CPB, E_pad = plan["NB"], plan["CPB"], plan["E_pad"]
    order, cum = plan["order"], plan["cum"]
    per_core = []
    for cr in plan["cores"]:
        n_lo, n_hi = cr["n_lo"], cr["n_hi"]
        x1s = np.zeros((E_pad, DIM), np.float32)
        x2s = np.zeros((E_pad, DIM), np.float32)
        idxf = np.zeros((E_pad, 1), np.float32)
        src = np.full(E_pad, -1, np.int64)
        pos = 0
        for b in range(NB):
            blo, bhi = n_lo + b * P, min(n_lo + (b + 1) * P, n_hi)
            se, ee = (int(cum[blo]), int(cum[bhi])) if blo < n_hi else (0, 0)
            sl = order[se:ee]
            cnt = ee - se
            x1s[pos:pos + cnt] = x1[sl]
            x2s[pos:pos + cnt] = x2[sl]
            idxf[pos:pos + cnt, 0] = (idxs[sl] - blo).astype(np.float32)
            src[pos:pos + cnt] = sl
            pos += P * int(CPB[b])
        per_core.append(dict(x1s=x1s, x2s=x2s, idxf=idxf, src=src))
    return per_core


def _build_WW(w3j, weights):
    """Packed block-diagonal WW per u-group: [288, 891] f32.
       Row (u,j) (u in group), local col (u-u_lo)*81 + k*9 + i."""
    ww3j = np.einsum("up,pijk->uijk", weights, w3j).astype(np.float32)
    GW = 11 * 81  # 891 cols max per group
    WW = np.zeros((DIM, GW), np.float32)
    for (u0, u1) in UGROUPS:
        for u in range(u0, u1):
            blk = ww3j[u].transpose(1, 2, 0).reshape(9, 81)  # [j,(k,i)]
            WW[u * 9:(u + 1) * 9, (u - u0) * 81:(u - u0 + 1) * 81] = blk
    return WW


# ----------------------------------------------------------------- device
def _build_nc(NB, CPB, E_pad):
    NBN = NB * P
    nc = bacc.Bacc("TRN2", target_bir_lowering=False, debug=False,
                   num_devices=NCORES)
    d_x1 = nc.dram_tensor("x1s", [E_pad, DIM], f32, kind="ExternalInput")
    d_x2 = nc.dram_tensor("x2s", [E_pad, DIM], f32, kind="ExternalInput")
    d_idxf = nc.dram_tensor("idxf", [E_pad, 1], f32, kind="ExternalInput")
    d_iota = nc.dram_tensor("iota", [P, P], f32, kind="ExternalInput")
    d_WW = nc.dram_tensor("WW", [DIM, 891], f32, kind="ExternalInput")
    d_out = nc.dram_tensor("out", [E_pad, DIM], f32, kind="ExternalOutput")

    chunk_of = []      # (block, chunk index) pairs in edge order
    for b in range(NB):
        for _ in range(int(CPB[b])):
            chunk_of.append(b)
    n_chunks = len(chunk_of)

    with tile.TileContext(nc) as tc:
        with tc.tile_pool(name="persist", bufs=1) as pp:
            iota_t = pp.tile([P, P], f32)
            nc.sync.dma_start(iota_t[:], d_iota[:])
            ident = pp.tile([P, P], f32)
            make_identity(nc, ident[:])
            # WW -> f32r tiles per group
            WWr = []
            for gi, (u0, u1) in enumerate(UGROUPS):
                r0, r1 = u0 * 9, u1 * 9
                gw = (u1 - u0) * 81
                stage = pp.tile([r1 - r0, 891], f32, tag="wwstage")
                nc.sync.dma_start(stage[:], d_WW[r0:r1, :])
                wr = pp.tile([r1 - r0, gw], f32r, tag=f"wwr{gi}")
                nc.scalar.copy(wr[:], stage[:, :gw])
                WWr.append(wr)
            # persistent tables
            tableT = []
            for gi, (u0, u1) in enumerate(UGROUPS):
                tT = pp.tile([u1 * 9 - u0 * 9, NBN], f32r, tag=f"tT{gi}")
                tableT.append(tT)
            Ctab = []
            for b in range(NB):
                ct = pp.tile([P, CDIM], f32r, tag=f"ct{b}")
                Ctab.append(ct)

            # ---------------- sweep 1: segment sum + transposes ----------
            with tc.tile_pool(name="s1", bufs=3) as s1, \
                 tc.tile_pool(name="s1p", bufs=2, space="PSUM") as s1p, \
                 tc.tile_pool(name="s1pt", bufs=2, space="PSUM") as s1pt:
                ci = 0
                for b in range(NB):
                    seg = s1p.tile([P, DIM], f32, tag="seg")
                    nch = int(CPB[b])
                    for k in range(nch):
                        c = ci + k
                        x2t = s1.tile([P, DIM], f32, tag="x2")
                        nc.sync.dma_start(x2t[:], d_x2[c * P:(c + 1) * P, :])
                        x2r = s1.tile([P, DIM], f32r, tag="x2r")
                        nc.scalar.copy(x2r[:], x2t[:])
                        idt = s1.tile([P, 1], f32, tag="id")
                        nc.sync.dma_start(idt[:], d_idxf[c * P:(c + 1) * P, :])
                        oh = s1.tile([P, P], f32r, tag="oh")
                        nc.vector.tensor_scalar(
                            out=oh[:], in0=iota_t[:], scalar1=idt[:, :1],
                            scalar2=None, op0=mybir.AluOpType.is_equal)
                        nc.tensor.matmul(seg[:], lhsT=oh[:], rhs=x2r[:],
                                         start=(k == 0), stop=(k == nch - 1))
                    ci += nch
                    # transpose table block into tableT groups (f32r)
                    tabs = s1.tile([P, DIM], f32, tag="tab")
                    nc.scalar.copy(tabs[:], seg[:])
                    for gi, (u0, u1) in enumerate(UGROUPS):
                        r0, r1 = u0 * 9, u1 * 9
                        tp = s1pt.tile([P, P], f32, tag="tp")
                        nc.tensor.transpose(tp[:r1 - r0, :], tabs[:, r0:r1],
                                            ident[:])
                        nc.scalar.copy(tableT[gi][:, b * P:(b + 1) * P],
                                       tp[:r1 - r0, :])

            # ---------------- mid: Ctable build --------------------------
            with tc.tile_pool(name="cb", bufs=2, space="PSUM") as cbp:
                for b in range(NB):
                    for gi, (u0, u1) in enumerate(UGROUPS):
                        gw = (u1 - u0) * 81
                        col0 = u0 * 81
                        # fp32r: even N, psum offset 0 -> overlapped 512 chunks
                        starts = [0] if gw <= 512 else [0, gw - 512]
                        for n0 in starts:
                            n1 = min(n0 + 512, gw)
                            acc = cbp.tile([P, 512], f32, tag="cb")
                            nc.tensor.matmul(
                                acc[:, :n1 - n0],
                                lhsT=tableT[gi][:, b * P:(b + 1) * P],
                                rhs=WWr[gi][:, n0:n1], start=True, stop=True)
                            nc.scalar.copy(
                                Ctab[b][:, col0 + n0:col0 + n1],
                                acc[:, :n1 - n0])

            # ---------------- sweep 2: gather + contract ------------------
            with tc.tile_pool(name="s2", bufs=3) as s2, \
                 tc.tile_pool(name="s2cg", bufs=4, space="PSUM") as s2cg, \
                 tc.tile_pool(name="s2tp", bufs=2, space="PSUM") as s2tp:
                for c in range(n_chunks):
                    b = chunk_of[c]
                    idt = s2.tile([P, 1], f32, tag="id")
                    nc.sync.dma_start(idt[:], d_idxf[c * P:(c + 1) * P, :])
                    oh = s2.tile([P, P], f32, tag="oh")
                    nc.vector.tensor_scalar(
                        out=oh[:], in0=iota_t[:], scalar1=idt[:, :1],
                        scalar2=None, op0=mybir.AluOpType.is_equal)
                    ohTp = s2tp.tile([P, P], f32, tag="ohT")
                    nc.tensor.transpose(ohTp[:], oh[:], ident[:])
                    ohT = s2.tile([P, P], f32r, tag="ohTr")
                    nc.scalar.copy(ohT[:], ohTp[:])
                    # CG = onehotT.T @ Ctable[b]  (PSUM, 2x 1296-col tiles)
                    x1b = s2.tile([P, DIM], bf16, tag="x1b")
                    nc.gpsimd.dma_start(x1b[:], d_x1[c * P:(c + 1) * P, :])
                    cgb = s2.tile([P, CDIM], bf16, tag="cgb")
                    for n0 in range(0, CDIM, 512):
                        n1 = min(n0 + 512, CDIM)
                        cg = s2cg.tile([P, 512], f32, tag="cg")
                        nc.tensor.matmul(
                            cg[:, :n1 - n0], lhsT=ohT[:],
                            rhs=Ctab[b][:, n0:n1], start=True, stop=True)
                        nc.scalar.copy(cgb[:, n0:n1], cg[:, :n1 - n0])
                    # T = x1 (bcast over k) * CG ; layout (u,k,i)
                    T = s2.tile([P, CDIM], bf16, tag="T")
                    x1_b = x1b[:].rearrange("p (u k i) -> p u k i",
                                            u=MUL, k=1, i=BD)
                    nc.vector.tensor_tensor(
                        out=T[:].rearrange("p (u k i) -> p u k i", u=MUL, k=BD),
                        in0=x1_b.to_broadcast([P, MUL, BD, BD]),
                        in1=cgb[:].rearrange("p (u k i) -> p u k i",
                                             u=MUL, k=BD),
                        op=mybir.AluOpType.mult)
                    # reduce over i (innermost) -> out [P, (u,k)] f32
                    outt = s2.tile([P, DIM], f32, tag="out")
                    T4 = T[:].rearrange("p (u k i) -> p u k i", u=MUL, k=BD)
                    nc.vector.tensor_reduce(
                        out=outt[:].rearrange("p (u k) -> p u k", u=MUL),
                        in_=T4, axis=mybir.AxisListType.X,
                        op=mybir.AluOpType.add)
                    nc.sync.dma_start(d_out[c * P:(c + 1) * P, :], outt[:])
    nc.compile()
    return nc


# ----------------------------------------------------------------- entry
def kernel(x1, x2, idxs, scatter_dim_size, w3j, weights):
    x1 = np.asarray(x1, dtype=np.float32)
    x2 = np.asarray(x2, dtype=np.float32)
    idxs_np = np.asarray(idxs).astype(np.int64)
    w3j = np.asarray(w3j, dtype=np.float32)
    weights = np.asarray(weights, dtype=np.float32)

    plan = _plan(idxs_np)
    per_core = _core_arrays(plan, idxs_np, x1, x2)
    WW = _build_WW(w3j, weights)
    iota = np.broadcast_to(np.arange(P, dtype=np.float32)[None, :],
                           (P, P)).copy()

    key = (plan["NB"], tuple(plan["CPB"]), plan["E_pad"])
    if key not in _CACHE:
        _CACHE[key] = _build_nc(plan["NB"], plan["CPB"], plan["E_pad"])
    nc = _CACHE[key]

    in_maps = [{"x1s": pc["x1s"], "x2s": pc["x2s"], "idxf": pc["idxf"],
                "iota": iota, "WW": WW} for pc in per_core]
    res = bass_utils.run_bass_kernel_spmd(nc, in_maps,
                                          core_ids=list(range(NCORES)))
    out = np.zeros((E, DIM), np.float32)
    for pc, r in zip(per_core, res.results):
        real = pc["src"] >= 0
        out[pc["src"][real]] = r["out"][real]
    return out.reshape(E, MUL, BD)


if __name__ == "__main__":
    rng = np.random.default_rng(0)
    sys.path.insert(0, "/root/problem")
    import reference as ref
    import jax
    with jax.default_device(jax.devices("cpu")[0]):
        inputs = {k: np.asarray(v) if hasattr(v, "shape") else v
                  for k, v in ref.setup_inputs().items()}
    got = kernel(**inputs)
    print("kernel done", got.shape)
